# revision 36
# baseline (speedup 1.0000x reference)
"""Trainium2 Bass kernel for GQA attention layer (B=1, T=2048, HID=4096,
32 q-heads / 8 kv-heads, head_dim 128, RoPE, causal) sharded over 8 cores.

Sharding: tensor-parallel over heads. Core c owns q-heads 4c..4c+3 and
kv-head c. Per-head attention outputs are AllGathered eagerly (16 chunks);
each core then computes a 512-row slice of the output projection over the
full 4096 hd dims, so no AllReduce is needed. Host assembles the slices.

Structure (vs previous version):
- projection runs in three passes (q, k, v) so the RoPE DVE chain hides
  under the v pass instead of stalling the first score matmuls, and the
  k accumulator's PSUM WAR wait resolves under the q pass
- attention processes q-heads in pairs: one fused exp per k-block spanning
  both heads' score banks ([128,2,512] PSUM tiles), softmax denominators
  via two col-tiled (M=32) matmuls that co-run in distinct PE col groups
- AllGather is per-head and eager; outproj phases are slotted between
  proj/attn phases to cover rope latency (outproj3 is head-major so it
  consumes gather chunks in arrival order)
"""

import numpy as np

import concourse.bacc as bacc
import concourse.mybir as mybir
import concourse.tile as tile
from concourse.bass_utils import run_bass_kernel_spmd
from concourse.tile import add_dep_helper

T = 2048
HID = 4096
D = 128
N_HEADS = 32
N_KV = 8
HQ = N_HEADS // N_KV  # q heads per core (=4)
TT = 512  # t tile
NTT = T // TT  # 4
NH = HID // 128  # 32 h-tiles
SCALE = 1.0 / np.sqrt(np.float32(D))
ROPE_BASE = 10000.0
N_CORES = 8

_F32 = mybir.dt.float32
_DT = mybir.dt.bfloat16

_cached = None


def _build():
    nc = bacc.Bacc("TRN2", target_bir_lowering=False, debug=False, num_devices=N_CORES)

    xT = nc.dram_tensor("xT", [HID, T], _DT, kind="ExternalInput").ap()
    wqkvT = nc.dram_tensor(
        "wqkvT", [HID, (HQ + 2) * D], _DT, kind="ExternalInput"
    ).ap()
    woT = nc.dram_tensor("woT", [HID, HQ * D], _DT, kind="ExternalInput").ap()
    cos2 = nc.dram_tensor("cos2", [128, T], _DT, kind="ExternalInput").ap()
    sinS = nc.dram_tensor("sinS", [128, T], _DT, kind="ExternalInput").ap()
    mask2 = nc.dram_tensor("mask2", [128, 4, 2, TT], _DT, kind="ExternalInput").ap()
    ones_i = nc.dram_tensor("ones_i", [128, 128], _DT, kind="ExternalInput").ap()
    ident_i = nc.dram_tensor("ident_i", [128, 128], _DT, kind="ExternalInput").ap()
    out = nc.dram_tensor("out", [HQ * D, T], _F32, kind="ExternalOutput").ap()

    Exp = mybir.ActivationFunctionType.Exp

    with tile.TileContext(nc) as tc:
        with (
            tc.tile_pool(name="const", bufs=1) as const,
            tc.tile_pool(name="big", bufs=1) as big,
            tc.tile_pool(name="sb", bufs=1) as sb,
            tc.tile_pool(name="ps", bufs=1, space="PSUM") as ps,
            tc.tile_pool(name="dram", bufs=1, space="DRAM") as dram,
        ):
            # ---- constants / persistent weights in SBUF ----
            cos_sb = const.tile([128, T], _DT, name="cos_sb")
            sin_sb = const.tile([128, T], _DT, name="sin_sb")
            # mask_sb[p, diag, hp, t_local]
            mask_sb = const.tile([128, 4, 2, TT], _DT, name="mask_sb")
            ones_sb = const.tile([128, 128], _DT, name="ones_sb")
            ident_sb = const.tile([128, 128], _DT, name="ident_sb")
            wqkv_t = [
                const.tile([128, (HQ + 2) * D], _DT, name=f"wqkv_t{j}")
                for j in range(NH)
            ]
            wo_sb = const.tile([128, NH * HQ * D], _DT, name="wo_sb")

            def emit_consts():
                # emitted after proj(0)'s weight/x DMA stream so the first
                # matmuls aren't queued behind 2MB of constants; all of these
                # are ungated so they drain without blocking their queues
                nc.scalar.dma_start(out=ident_sb[:], in_=ident_i[:])
                nc.scalar.dma_start(out=cos_sb[:], in_=cos2[:])
                nc.gpsimd.dma_start(out=sin_sb[:], in_=sinS[:])
                nc.gpsimd.dma_start(out=ones_sb[:], in_=ones_i[:])
                nc.gpsimd.dma_start(out=mask_sb[:], in_=mask2[:])

            qrot = [big.tile([128, T], _DT, name=f"qrot{h}") for h in range(HQ)]
            krot = big.tile([128, T], _DT, name="krot")
            v_sb = big.tile([128, T], _DT, name="v_sb")  # V[s,d]: block k at cols 128k

            attn_local = [
                dram.tile([HQ * D, TT], _DT, name=f"attn_local{i}") for i in range(NTT)
            ]
            # per-tile gathered activations: [core*512 rows, TT]
            attn_full = [
                dram.tile(
                    [N_CORES * HQ * D, TT],
                    _DT,
                    addr_space="Shared",
                    name=f"attn_full{i}",
                )
                for i in range(NTT)
            ]
            # tile 3 is gathered in two half-chunks so outproj(3) can start
            # on heads 0-1 while heads 2-3 are still in flight
            attn_half = [
                dram.tile(
                    [N_CORES * 2 * D, TT], _DT, addr_space="Shared", name=f"attn_h{i}"
                )
                for i in range(2)
            ]

            def proj(ti, post_dma=None):
                tsl = slice(TT * ti, TT * (ti + 1))
                q01 = ps.tile([128, 2, TT], _F32, tag="sc", bufs=2, name=f"q01_{ti}")
                q23 = ps.tile([128, 2, TT], _F32, tag="sc", bufs=2, name=f"q23_{ti}")
                k_ps = ps.tile([128, TT], _F32, tag="at0", name=f"k_ps{ti}")
                xts = []
                # ---- q pass (4 MMs per h-tile) ----
                for hi in range(NH):
                    hsl = slice(128 * hi, 128 * (hi + 1))
                    # weight preloads are ungated: they drain early without
                    # blocking the scalar queue ahead of the attention exps
                    weng = nc.gpsimd if hi % 2 == 0 else nc.scalar
                    if ti == 0:
                        weng.dma_start(out=wqkv_t[hi][:], in_=wqkvT[hsl, :])
                    elif ti == 1:
                        weng.dma_start(
                            out=wo_sb[:, 512 * hi : 512 * (hi + 1)], in_=woT[hsl, :]
                        )
                    xt = sb.tile([128, TT], _DT, tag="x", bufs=32)
                    # x tiles are slot-WAR gated: keep them all on sync so a
                    # waiting head-of-queue DMA never starves the exps
                    nc.sync.dma_start(out=xt[:], in_=xT[hsl, tsl])
                    xts.append(xt)
                    st, sp = hi == 0, hi == NH - 1
                    for h in range(HQ):
                        nc.tensor.matmul(
                            (q01 if h < 2 else q23)[:, h % 2, :],
                            wqkv_t[hi][:, 128 * h : 128 * (h + 1)],
                            xt[:],
                            start=st,
                            stop=sp,
                        )
                if post_dma is not None:
                    post_dma()
                # ---- k pass ----
                for hi in range(NH):
                    nc.tensor.matmul(
                        k_ps[:],
                        wqkv_t[hi][:, HQ * D : (HQ + 1) * D],
                        xts[hi][:],
                        start=hi == 0,
                        stop=hi == NH - 1,
                    )

                # ---- RoPE (DVE + swap DMAs), overlapped with the next
                # outproj phase ----
                order = [HQ, 0, 1, 2, 3]  # k first: scores h0 need krot+qrot0
                stage = {}

                def src_of(h):
                    if h == HQ:
                        return k_ps[:]
                    return (q01 if h < 2 else q23)[:, h % 2, :]

                def rope_front(h):
                    qf = sb.tile([128, TT], _DT, tag="qf", bufs=5)
                    nc.vector.tensor_copy(qf[:], src_of(h))
                    qs = sb.tile([128, TT], _DT, tag="qs", bufs=5)
                    # gpsimd queue carries no bulk traffic -> low latency
                    nc.gpsimd.dma_start(out=qs[0:64, :], in_=qf[64:128, :])
                    nc.gpsimd.dma_start(out=qs[64:128, :], in_=qf[0:64, :])
                    t1 = sb.tile([128, TT], _DT, tag="t1", bufs=5)
                    nc.vector.tensor_mul(t1[:], qf[:], cos_sb[:, tsl])
                    stage[h] = (qs, t1)

                def rope_back(h):
                    qs, t1 = stage.pop(h)
                    t2 = sb.tile([128, TT], _DT, tag="t2", bufs=2)
                    nc.vector.tensor_mul(t2[:], qs[:], sin_sb[:, tsl])
                    dst = qrot[h][:, tsl] if h < HQ else krot[:, tsl]
                    nc.vector.tensor_add(dst, t1[:], t2[:])

                # all fronts first: batches the casts + swap DMAs so the
                # engine->DMA->engine latency is paid once, not per head
                for h in order:
                    rope_front(h)
                for h in order:
                    rope_back(h)

                # ---- v pass + transpose to [s,d] blocks (emitted after the
                # rope so the rope's DVE ops win scheduler priority) ----
                vT_ps = ps.tile([128, TT], _F32, tag="den", name=f"vT_ps{ti}")
                for hi in range(NH):
                    nc.tensor.matmul(
                        vT_ps[:],
                        wqkv_t[hi][:, (HQ + 1) * D : (HQ + 2) * D],
                        xts[hi][:],
                        start=hi == 0,
                        stop=hi == NH - 1,
                    )
                vT_sb = sb.tile([128, TT], _DT, tag="vTs", bufs=1)
                nc.vector.tensor_copy(vT_sb[:], vT_ps[:])
                for j in range(TT // 128):
                    vtr = ps.tile([128, 128], _DT, tag="tr", bufs=1)
                    nc.tensor.transpose(
                        vtr[:], vT_sb[:, 128 * j : 128 * (j + 1)], ident_sb[:]
                    )
                    k = (TT // 128) * ti + j
                    nc.vector.tensor_copy(v_sb[:, 128 * k : 128 * (k + 1)], vtr[:])

            def attn(ti):
                nblk = (TT // 128) * (ti + 1)
                last_exp = [None]
                for pair in range(2):
                    heads = (2 * pair, 2 * pair + 1)
                    at = [
                        ps.tile(
                            [128, TT], _F32, tag=f"at{hp}", name=f"at{ti}_{pair}{hp}"
                        )
                        for hp in range(2)
                    ]
                    den = ps.tile([128, TT], _F32, tag="den", name=f"den{ti}_{pair}")
                    def lo_of(k):
                        diag = k - (TT // 128) * ti
                        return 128 * diag if diag > 0 else 0

                    def emit_sc(k):
                        # scores for both heads + fused exp (+ causal mask)
                        diag = k - (TT // 128) * ti
                        lo = lo_of(k)
                        qsl = slice(TT * ti + lo, TT * (ti + 1))
                        ksl = slice(128 * k, 128 * (k + 1))
                        sc2 = ps.tile(
                            [128, 2, TT], _F32, tag="sc", bufs=2, name=f"sc{ti}_{k}"
                        )
                        for hp, h in enumerate(heads):
                            nc.tensor.matmul(
                                sc2[:, hp, lo:TT],
                                krot[:, ksl],
                                qrot[h][:, qsl],
                                start=True,
                                stop=True,
                            )
                        probs2 = sb.tile([128, 2, TT], _DT, tag="pr", bufs=5)
                        if diag >= 0:
                            ptmp = sb.tile([128, 2, TT], _DT, tag="pt", bufs=2)
                            ex = nc.scalar.activation(
                                ptmp[:, :, lo:TT], sc2[:, :, lo:TT], Exp, scale=SCALE
                            )
                            nc.vector.tensor_mul(
                                probs2[:, :, lo:TT],
                                ptmp[:, :, lo:TT],
                                mask_sb[:, diag, :, lo:TT],
                            )
                        else:
                            ex = nc.scalar.activation(
                                probs2[:, :, lo:TT], sc2[:, :, lo:TT], Exp, scale=SCALE
                            )
                        last_exp[0] = ex
                        return probs2

                    # 2-block software pipeline: scores+exp run ahead of pv/den
                    probs_t = {}
                    for k in range(min(2, nblk)):
                        probs_t[k] = emit_sc(k)
                    for k in range(nblk):
                        if k + 2 < nblk:
                            probs_t[k + 2] = emit_sc(k + 2)
                        probs2 = probs_t.pop(k)
                        lo = lo_of(k)
                        ksl = slice(128 * k, 128 * (k + 1))
                        st, sp = k == 0, k == nblk - 1
                        for hp, h in enumerate(heads):
                            nc.tensor.matmul(
                                den[32 * hp : 32 * (hp + 1), lo:TT],
                                ones_sb[:, 0:32],
                                probs2[:, hp, lo:TT],
                                start=st,
                                stop=sp,
                                tile_position=(0, 32 * hp),
                                # interp's zero-region group check rejects two
                                # col-tiled groups in one bank; HW-verified OK
                                skip_group_check=True,
                            )
                        for hp, h in enumerate(heads):
                            nc.tensor.matmul(
                                at[hp][:, lo:TT],
                                v_sb[:, ksl],
                                probs2[:, hp, lo:TT],
                                start=st,
                                stop=sp,
                            )
                    # normalize + store + eager per-head AllGather
                    # full-partition recip of the den bank (rows outside the
                    # pair's 32-row slices are garbage and never read)
                    r1 = sb.tile([128, TT], _F32, tag="r1", bufs=1)
                    nc.vector.reciprocal_approx_fast(r1[:, :], den[:, :])
                    for hp, h in enumerate(heads):
                        psl = slice(32 * hp, 32 * hp + 1)
                        # partition_broadcast only works from partition 0:
                        # hp=0 sits there already; hp=1 hops down via DMA
                        if hp > 0:
                            r0 = sb.tile([128, TT], _F32, tag="r0", bufs=2)
                            nc.gpsimd.dma_start(out=r0[0:1, :], in_=r1[psl, :])
                            bsrc = r0[0:1, :]
                        else:
                            bsrc = r1[0:1, :]
                        rB = sb.tile([128, TT], _F32, tag="rB", bufs=2)
                        nc.gpsimd.partition_broadcast(rB[:], bsrc, channels=128)
                        anorm = sb.tile([128, TT], _DT, tag="an", bufs=1)
                        nc.vector.tensor_mul(anorm[:], at[hp][:], rB[:])
                        nc.gpsimd.dma_start(
                            out=attn_local[ti][128 * h : 128 * (h + 1), :],
                            in_=anorm[:],
                        )
                    if ti == NTT - 1:
                        # last tile: gather each pair-half immediately
                        nc.gpsimd.collective_compute(
                            "AllGather",
                            mybir.AluOpType.bypass,
                            replica_groups=[list(range(N_CORES))],
                            ins=[attn_local[ti][256 * pair : 256 * (pair + 1), :]],
                            outs=[attn_half[pair].opt()],
                        )
                if ti < NTT - 1:
                    nc.gpsimd.collective_compute(
                        "AllGather",
                        mybir.AluOpType.bypass,
                        replica_groups=[list(range(N_CORES))],
                        ins=[attn_local[ti].opt()],
                        outs=[attn_full[ti].opt()],
                    )
                return last_exp[0]

            def outproj(ti, h_major=False, delay_after=None):
                if h_major:
                    order = [4 * r + h for h in range(HQ) for r in range(N_CORES)]
                else:
                    order = list(range(NH))
                o01 = ps.tile([128, 2, TT], _F32, tag="sc", bufs=2, name=f"o01_{ti}")
                o23 = ps.tile([128, 2, TT], _F32, tag="sc", bufs=2, name=f"o23_{ti}")
                for idx, j in enumerate(order):
                    r, h = j // HQ, j % HQ
                    ag = sb.tile([128, TT], _DT, tag="ag", bufs=8)
                    if h_major:
                        row = 256 * r + 128 * (h % 2)
                        src = attn_half[h // 2][row : row + 128, :]
                    else:
                        src = attn_full[ti][128 * j : 128 * (j + 1), :]
                    # split the gathered-activation stream over two queues
                    eng = nc.sync if idx % 2 == 0 else nc.scalar
                    dma = eng.dma_start(out=ag[:], in_=src)
                    if idx < 2 and delay_after is not None:
                        # don't let the scheduler hoist the collective-gated
                        # reads ahead of earlier phases' queue traffic
                        add_dep_helper(
                            dma.ins, delay_after.ins, sync=True, reason="ag-delay"
                        )
                    st, sp = idx == 0, idx == NH - 1
                    for o in range(4):
                        nc.tensor.matmul(
                            (o01 if o < 2 else o23)[:, o % 2, :],
                            wo_sb[:, 512 * j + 128 * o : 512 * j + 128 * (o + 1)],
                            ag[:],
                            start=st,
                            stop=sp,
                        )
                for o in range(4):
                    oc = sb.tile([128, TT], _F32, tag="oc", bufs=2)
                    nc.vector.tensor_copy(oc[:], (o01 if o < 2 else o23)[:, o % 2, :])
                    nc.gpsimd.dma_start(
                        out=out[128 * o : 128 * (o + 1), TT * ti : TT * (ti + 1)],
                        in_=oc[:],
                    )

            proj(0, emit_consts)
            attn(0)
            proj(1)
            e1 = attn(1)
            proj(2)
            outproj(0, delay_after=e1)
            e2 = attn(2)
            proj(3)
            outproj(1, delay_after=e2)
            attn(3)
            outproj(2)
            outproj(3, h_major=True)

    nc.compile()
    return nc


def _host_inputs(hidden_states, Wq, Wk, Wv, Wo):
    import ml_dtypes

    bf16 = ml_dtypes.bfloat16
    x = np.asarray(hidden_states, dtype=np.float32).reshape(T, HID)
    xT = np.ascontiguousarray(x.T).astype(bf16)

    pos = np.arange(T, dtype=np.float32)
    inv_freq = ROPE_BASE ** (-np.arange(0, D, 2, dtype=np.float32) / D)  # [64]
    ang = pos[:, None] * inv_freq[None, :]  # [T, 64]
    cosT = np.cos(ang).T.astype(np.float32)  # [64, T]
    sinT = np.sin(ang).T.astype(np.float32)
    cos2 = np.ascontiguousarray(np.concatenate([cosT, cosT], axis=0))
    sinS = np.ascontiguousarray(np.concatenate([-sinT, sinT], axis=0))

    p = np.arange(128)[:, None]
    tp = np.arange(TT)[None, :]
    # mask2[p, diag, hp, t] = p <= t - 128*diag, duplicated over hp
    mask2 = np.stack(
        [
            np.stack([(p <= tp - 128 * j).astype(np.float32)] * 2, axis=1)
            for j in range(4)
        ],
        axis=1,
    )  # [128, 4, 2, TT]
    mask2 = np.ascontiguousarray(mask2).astype(bf16)
    ones = np.ones((128, 128), dtype=bf16)
    ident = np.eye(128, dtype=np.float32).astype(bf16)

    Wq = np.asarray(Wq, dtype=np.float32)
    Wk = np.asarray(Wk, dtype=np.float32)
    Wv = np.asarray(Wv, dtype=np.float32)
    Wo = np.asarray(Wo, dtype=np.float32)

    in_maps = []
    for c in range(N_CORES):
        qs = slice(HQ * D * c, HQ * D * (c + 1))
        ks = slice(D * c, D * (c + 1))
        in_maps.append(
            {
                "xT": xT,
                "wqkvT": np.ascontiguousarray(
                    np.concatenate(
                        [Wq[qs, :].T, Wk[ks, :].T, Wv[ks, :].T], axis=1
                    )
                ).astype(bf16),
                "woT": np.ascontiguousarray(Wo[qs, :].T).astype(bf16),
                "cos2": cos2.astype(bf16),
                "sinS": sinS.astype(bf16),
                "mask2": mask2,
                "ones_i": ones,
                "ident_i": ident,
            }
        )
    return in_maps


def get_program():
    global _cached
    if _cached is None:
        _cached = _build()
    return _cached


def kernel(hidden_states, Wq, Wk, Wv, Wo):
    nc = get_program()
    in_maps = _host_inputs(hidden_states, Wq, Wk, Wv, Wo)
    res = run_bass_kernel_spmd(nc, in_maps, list(range(N_CORES)))
    outT = np.concatenate([res.results[c]["out"] for c in range(N_CORES)], axis=0)
    return np.ascontiguousarray(outT.T).reshape(1, T, HID).astype(np.float32)


# revision 37
# speedup vs baseline: 1.0669x; 1.0669x over previous
"""Trainium2 Bass kernel for GQA attention layer (B=1, T=2048, HID=4096,
32 q-heads / 8 kv-heads, head_dim 128, RoPE, causal) sharded over 8 cores.

Sharding: tensor-parallel over heads. Core c owns q-heads 4c..4c+3 and
kv-head c. Per-head attention outputs are AllGathered eagerly (16 chunks);
each core then computes a 512-row slice of the output projection over the
full 4096 hd dims, so no AllReduce is needed. Host assembles the slices.

Structure (vs previous version):
- projection runs in three passes (q, k, v) so the RoPE DVE chain hides
  under the v pass instead of stalling the first score matmuls, and the
  k accumulator's PSUM WAR wait resolves under the q pass
- attention processes q-heads in pairs: one fused exp per k-block spanning
  both heads' score banks ([128,2,512] PSUM tiles), softmax denominators
  via two col-tiled (M=32) matmuls that co-run in distinct PE col groups
- AllGather is per-head and eager; outproj phases are slotted between
  proj/attn phases to cover rope latency (outproj3 is head-major so it
  consumes gather chunks in arrival order)
"""

import numpy as np

import concourse.bacc as bacc
import concourse.mybir as mybir
import concourse.tile as tile
from concourse.bass_utils import run_bass_kernel_spmd
from concourse.tile import add_dep_helper

T = 2048
HID = 4096
D = 128
N_HEADS = 32
N_KV = 8
HQ = N_HEADS // N_KV  # q heads per core (=4)
TT = 512  # t tile
NTT = T // TT  # 4
NH = HID // 128  # 32 h-tiles
SCALE = 1.0 / np.sqrt(np.float32(D))
ROPE_BASE = 10000.0
N_CORES = 8

_F32 = mybir.dt.float32
_DT = mybir.dt.bfloat16

_cached = None


def _build():
    nc = bacc.Bacc("TRN2", target_bir_lowering=False, debug=False, num_devices=N_CORES)

    xT = nc.dram_tensor("xT", [HID, T], _DT, kind="ExternalInput").ap()
    wqkvT = nc.dram_tensor(
        "wqkvT", [HID, (HQ + 2) * D], _DT, kind="ExternalInput"
    ).ap()
    woT = nc.dram_tensor("woT", [HID, HQ * D], _DT, kind="ExternalInput").ap()
    cos2 = nc.dram_tensor("cos2", [128, T], _DT, kind="ExternalInput").ap()
    sinS = nc.dram_tensor("sinS", [128, T], _DT, kind="ExternalInput").ap()
    mask2 = nc.dram_tensor("mask2", [128, 4, 2, TT], _DT, kind="ExternalInput").ap()
    ones_i = nc.dram_tensor("ones_i", [128, 128], _DT, kind="ExternalInput").ap()
    ident_i = nc.dram_tensor("ident_i", [128, 128], _DT, kind="ExternalInput").ap()
    out = nc.dram_tensor("out", [HQ * D, T], _F32, kind="ExternalOutput").ap()

    Exp = mybir.ActivationFunctionType.Exp

    with tile.TileContext(nc) as tc:
        with (
            tc.tile_pool(name="const", bufs=1) as const,
            tc.tile_pool(name="big", bufs=1) as big,
            tc.tile_pool(name="sb", bufs=1) as sb,
            tc.tile_pool(name="ps", bufs=1, space="PSUM") as ps,
            tc.tile_pool(name="dram", bufs=1, space="DRAM") as dram,
        ):
            # ---- constants / persistent weights in SBUF ----
            cos_sb = const.tile([128, T], _DT, name="cos_sb")
            sin_sb = const.tile([128, T], _DT, name="sin_sb")
            # mask_sb[p, diag, hp, t_local]
            mask_sb = const.tile([128, 4, 2, TT], _DT, name="mask_sb")
            ones_sb = const.tile([128, 128], _DT, name="ones_sb")
            ident_sb = const.tile([128, 128], _DT, name="ident_sb")
            wqkv_t = [
                const.tile([128, (HQ + 2) * D], _DT, name=f"wqkv_t{j}")
                for j in range(NH)
            ]
            wo_sb = const.tile([128, NH * HQ * D], _DT, name="wo_sb")

            def emit_consts():
                # emitted after proj(0)'s weight/x DMA stream so the first
                # matmuls aren't queued behind 2MB of constants; all of these
                # are ungated so they drain without blocking their queues
                nc.scalar.dma_start(out=ident_sb[:], in_=ident_i[:])
                nc.scalar.dma_start(out=cos_sb[:], in_=cos2[:])
                nc.gpsimd.dma_start(out=sin_sb[:], in_=sinS[:])
                nc.gpsimd.dma_start(out=ones_sb[:], in_=ones_i[:])
                nc.gpsimd.dma_start(out=mask_sb[:], in_=mask2[:])

            qrot = [big.tile([128, T], _DT, name=f"qrot{h}") for h in range(HQ)]
            krot = big.tile([128, T], _DT, name="krot")
            v_sb = big.tile([128, T], _DT, name="v_sb")  # V[s,d]: block k at cols 128k

            attn_local = [
                dram.tile([HQ * D, TT], _DT, name=f"attn_local{i}") for i in range(NTT)
            ]
            # per-tile gathered activations: [core*512 rows, TT]
            attn_full = [
                dram.tile(
                    [N_CORES * HQ * D, TT],
                    _DT,
                    addr_space="Shared",
                    name=f"attn_full{i}",
                )
                for i in range(NTT)
            ]
            # tile 3 is gathered in two half-chunks so outproj(3) can start
            # on heads 0-1 while heads 2-3 are still in flight
            attn_half = [
                dram.tile(
                    [N_CORES * 2 * D, TT], _DT, addr_space="Shared", name=f"attn_h{i}"
                )
                for i in range(2)
            ]

            # PE warmup: dummy matmuls on (not yet loaded) const tiles fill
            # the DMA-bound first ~10us and flip the HAM clock gate to 8/8
            # before the real matmuls arrive. Results are discarded.
            warm = ps.tile([128, 2, TT], _F32, tag="sc", bufs=2, name="warm")
            for _ in range(15):
                nc.tensor.matmul(
                    warm[:, 0, :],
                    cos_sb[:, 0:128],
                    cos_sb[:, 512:1024],
                    start=True,
                    stop=True,
                )

            def proj(ti, post_dma=None):
                tsl = slice(TT * ti, TT * (ti + 1))
                q01 = ps.tile([128, 2, TT], _F32, tag="sc", bufs=2, name=f"q01_{ti}")
                q23 = ps.tile([128, 2, TT], _F32, tag="sc", bufs=2, name=f"q23_{ti}")
                k_ps = ps.tile([128, TT], _F32, tag="at0", name=f"k_ps{ti}")
                xts = []
                # ---- q pass (4 MMs per h-tile) ----
                for hi in range(NH):
                    hsl = slice(128 * hi, 128 * (hi + 1))
                    # weight preloads are ungated: they drain early without
                    # blocking the scalar queue ahead of the attention exps
                    weng = nc.gpsimd if hi % 2 == 0 else nc.scalar
                    if ti == 0:
                        weng.dma_start(out=wqkv_t[hi][:], in_=wqkvT[hsl, :])
                    elif ti == 1:
                        weng.dma_start(
                            out=wo_sb[:, 512 * hi : 512 * (hi + 1)], in_=woT[hsl, :]
                        )
                    xt = sb.tile([128, TT], _DT, tag="x", bufs=32)
                    # x tiles are slot-WAR gated: keep them all on sync so a
                    # waiting head-of-queue DMA never starves the exps
                    nc.sync.dma_start(out=xt[:], in_=xT[hsl, tsl])
                    xts.append(xt)
                    st, sp = hi == 0, hi == NH - 1
                    for h in range(HQ):
                        nc.tensor.matmul(
                            (q01 if h < 2 else q23)[:, h % 2, :],
                            wqkv_t[hi][:, 128 * h : 128 * (h + 1)],
                            xt[:],
                            start=st,
                            stop=sp,
                        )
                if post_dma is not None:
                    post_dma()
                # ---- k pass ----
                for hi in range(NH):
                    nc.tensor.matmul(
                        k_ps[:],
                        wqkv_t[hi][:, HQ * D : (HQ + 1) * D],
                        xts[hi][:],
                        start=hi == 0,
                        stop=hi == NH - 1,
                    )

                # ---- RoPE (DVE + swap DMAs), overlapped with the next
                # outproj phase ----
                order = [HQ, 0, 1, 2, 3]  # k first: scores h0 need krot+qrot0
                stage = {}

                def src_of(h):
                    if h == HQ:
                        return k_ps[:]
                    return (q01 if h < 2 else q23)[:, h % 2, :]

                def rope_front(h):
                    qf = sb.tile([128, TT], _DT, tag="qf", bufs=5)
                    nc.vector.tensor_copy(qf[:], src_of(h))
                    qs = sb.tile([128, TT], _DT, tag="qs", bufs=5)
                    # gpsimd queue carries no bulk traffic -> low latency
                    nc.gpsimd.dma_start(out=qs[0:64, :], in_=qf[64:128, :])
                    nc.gpsimd.dma_start(out=qs[64:128, :], in_=qf[0:64, :])
                    t1 = sb.tile([128, TT], _DT, tag="t1", bufs=5)
                    nc.vector.tensor_mul(t1[:], qf[:], cos_sb[:, tsl])
                    stage[h] = (qs, t1)

                def rope_back(h):
                    qs, t1 = stage.pop(h)
                    t2 = sb.tile([128, TT], _DT, tag="t2", bufs=2)
                    nc.vector.tensor_mul(t2[:], qs[:], sin_sb[:, tsl])
                    dst = qrot[h][:, tsl] if h < HQ else krot[:, tsl]
                    nc.vector.tensor_add(dst, t1[:], t2[:])

                # all fronts first: batches the casts + swap DMAs so the
                # engine->DMA->engine latency is paid once, not per head
                for h in order:
                    rope_front(h)
                for h in order:
                    rope_back(h)

                # ---- v pass + transpose to [s,d] blocks (emitted after the
                # rope so the rope's DVE ops win scheduler priority) ----
                vT_ps = ps.tile([128, TT], _F32, tag="den", name=f"vT_ps{ti}")
                for hi in range(NH):
                    nc.tensor.matmul(
                        vT_ps[:],
                        wqkv_t[hi][:, (HQ + 1) * D : (HQ + 2) * D],
                        xts[hi][:],
                        start=hi == 0,
                        stop=hi == NH - 1,
                    )
                vT_sb = sb.tile([128, TT], _DT, tag="vTs", bufs=1)
                nc.vector.tensor_copy(vT_sb[:], vT_ps[:])
                for j in range(TT // 128):
                    vtr = ps.tile([128, 128], _DT, tag="tr", bufs=1)
                    nc.tensor.transpose(
                        vtr[:], vT_sb[:, 128 * j : 128 * (j + 1)], ident_sb[:]
                    )
                    k = (TT // 128) * ti + j
                    nc.vector.tensor_copy(v_sb[:, 128 * k : 128 * (k + 1)], vtr[:])

            def attn(ti):
                nblk = (TT // 128) * (ti + 1)
                last_exp = [None]
                for pair in range(2):
                    heads = (2 * pair, 2 * pair + 1)
                    at = [
                        ps.tile(
                            [128, TT], _F32, tag=f"at{hp}", name=f"at{ti}_{pair}{hp}"
                        )
                        for hp in range(2)
                    ]
                    den = ps.tile([128, TT], _F32, tag="den", name=f"den{ti}_{pair}")
                    def lo_of(k):
                        diag = k - (TT // 128) * ti
                        return 128 * diag if diag > 0 else 0

                    def emit_sc(k):
                        # scores for both heads + fused exp (+ causal mask)
                        diag = k - (TT // 128) * ti
                        lo = lo_of(k)
                        qsl = slice(TT * ti + lo, TT * (ti + 1))
                        ksl = slice(128 * k, 128 * (k + 1))
                        sc2 = ps.tile(
                            [128, 2, TT], _F32, tag="sc", bufs=2, name=f"sc{ti}_{k}"
                        )
                        for hp, h in enumerate(heads):
                            nc.tensor.matmul(
                                sc2[:, hp, lo:TT],
                                krot[:, ksl],
                                qrot[h][:, qsl],
                                start=True,
                                stop=True,
                            )
                        probs2 = sb.tile([128, 2, TT], _DT, tag="pr", bufs=5)
                        if diag >= 0:
                            ptmp = sb.tile([128, 2, TT], _DT, tag="pt", bufs=2)
                            ex = nc.scalar.activation(
                                ptmp[:, :, lo:TT], sc2[:, :, lo:TT], Exp, scale=SCALE
                            )
                            nc.vector.tensor_mul(
                                probs2[:, :, lo:TT],
                                ptmp[:, :, lo:TT],
                                mask_sb[:, diag, :, lo:TT],
                            )
                        else:
                            ex = nc.scalar.activation(
                                probs2[:, :, lo:TT], sc2[:, :, lo:TT], Exp, scale=SCALE
                            )
                        last_exp[0] = ex
                        return probs2

                    # 2-block software pipeline: scores+exp run ahead of pv/den
                    probs_t = {}
                    for k in range(min(2, nblk)):
                        probs_t[k] = emit_sc(k)
                    for k in range(nblk):
                        if k + 2 < nblk:
                            probs_t[k + 2] = emit_sc(k + 2)
                        probs2 = probs_t.pop(k)
                        lo = lo_of(k)
                        ksl = slice(128 * k, 128 * (k + 1))
                        st, sp = k == 0, k == nblk - 1
                        for hp, h in enumerate(heads):
                            nc.tensor.matmul(
                                den[32 * hp : 32 * (hp + 1), lo:TT],
                                ones_sb[:, 0:32],
                                probs2[:, hp, lo:TT],
                                start=st,
                                stop=sp,
                                tile_position=(0, 32 * hp),
                                # interp's zero-region group check rejects two
                                # col-tiled groups in one bank; HW-verified OK
                                skip_group_check=True,
                            )
                        for hp, h in enumerate(heads):
                            nc.tensor.matmul(
                                at[hp][:, lo:TT],
                                v_sb[:, ksl],
                                probs2[:, hp, lo:TT],
                                start=st,
                                stop=sp,
                            )
                    # normalize + store + eager per-head AllGather
                    # full-partition recip of the den bank (rows outside the
                    # pair's 32-row slices are garbage and never read)
                    r1 = sb.tile([128, TT], _F32, tag="r1", bufs=1)
                    nc.vector.reciprocal_approx_fast(r1[:, :], den[:, :])
                    for hp, h in enumerate(heads):
                        psl = slice(32 * hp, 32 * hp + 1)
                        # partition_broadcast only works from partition 0:
                        # hp=0 sits there already; hp=1 hops down via DMA
                        if hp > 0:
                            r0 = sb.tile([128, TT], _F32, tag="r0", bufs=2)
                            nc.gpsimd.dma_start(out=r0[0:1, :], in_=r1[psl, :])
                            bsrc = r0[0:1, :]
                        else:
                            bsrc = r1[0:1, :]
                        rB = sb.tile([128, TT], _F32, tag="rB", bufs=2)
                        nc.gpsimd.partition_broadcast(rB[:], bsrc, channels=128)
                        anorm = sb.tile([128, TT], _DT, tag="an", bufs=1)
                        nc.vector.tensor_mul(anorm[:], at[hp][:], rB[:])
                        nc.gpsimd.dma_start(
                            out=attn_local[ti][128 * h : 128 * (h + 1), :],
                            in_=anorm[:],
                        )
                    if ti == NTT - 1:
                        # last tile: gather each pair-half immediately
                        nc.gpsimd.collective_compute(
                            "AllGather",
                            mybir.AluOpType.bypass,
                            replica_groups=[list(range(N_CORES))],
                            ins=[attn_local[ti][256 * pair : 256 * (pair + 1), :]],
                            outs=[attn_half[pair].opt()],
                        )
                if ti < NTT - 1:
                    nc.gpsimd.collective_compute(
                        "AllGather",
                        mybir.AluOpType.bypass,
                        replica_groups=[list(range(N_CORES))],
                        ins=[attn_local[ti].opt()],
                        outs=[attn_full[ti].opt()],
                    )
                return last_exp[0]

            def outproj(ti, h_major=False, delay_after=None):
                if h_major:
                    order = [4 * r + h for h in range(HQ) for r in range(N_CORES)]
                else:
                    order = list(range(NH))
                o01 = ps.tile([128, 2, TT], _F32, tag="sc", bufs=2, name=f"o01_{ti}")
                o23 = ps.tile([128, 2, TT], _F32, tag="sc", bufs=2, name=f"o23_{ti}")
                for idx, j in enumerate(order):
                    r, h = j // HQ, j % HQ
                    ag = sb.tile([128, TT], _DT, tag="ag", bufs=8)
                    if h_major:
                        row = 256 * r + 128 * (h % 2)
                        src = attn_half[h // 2][row : row + 128, :]
                    else:
                        src = attn_full[ti][128 * j : 128 * (j + 1), :]
                    # split the gathered-activation stream over two queues
                    eng = nc.sync if idx % 2 == 0 else nc.scalar
                    dma = eng.dma_start(out=ag[:], in_=src)
                    if idx < 2 and delay_after is not None:
                        # don't let the scheduler hoist the collective-gated
                        # reads ahead of earlier phases' queue traffic
                        add_dep_helper(
                            dma.ins, delay_after.ins, sync=True, reason="ag-delay"
                        )
                    st, sp = idx == 0, idx == NH - 1
                    for o in range(4):
                        nc.tensor.matmul(
                            (o01 if o < 2 else o23)[:, o % 2, :],
                            wo_sb[:, 512 * j + 128 * o : 512 * j + 128 * (o + 1)],
                            ag[:],
                            start=st,
                            stop=sp,
                        )
                for o in range(4):
                    oc = sb.tile([128, TT], _F32, tag="oc", bufs=2)
                    nc.vector.tensor_copy(oc[:], (o01 if o < 2 else o23)[:, o % 2, :])
                    nc.gpsimd.dma_start(
                        out=out[128 * o : 128 * (o + 1), TT * ti : TT * (ti + 1)],
                        in_=oc[:],
                    )

            proj(0, emit_consts)
            attn(0)
            proj(1)
            e1 = attn(1)
            proj(2)
            outproj(0, delay_after=e1)
            e2 = attn(2)
            proj(3)
            outproj(1, delay_after=e2)
            attn(3)
            outproj(2)
            outproj(3, h_major=True)

    nc.compile()
    return nc


def _host_inputs(hidden_states, Wq, Wk, Wv, Wo):
    import ml_dtypes

    bf16 = ml_dtypes.bfloat16
    x = np.asarray(hidden_states, dtype=np.float32).reshape(T, HID)
    xT = np.ascontiguousarray(x.T).astype(bf16)

    pos = np.arange(T, dtype=np.float32)
    inv_freq = ROPE_BASE ** (-np.arange(0, D, 2, dtype=np.float32) / D)  # [64]
    ang = pos[:, None] * inv_freq[None, :]  # [T, 64]
    cosT = np.cos(ang).T.astype(np.float32)  # [64, T]
    sinT = np.sin(ang).T.astype(np.float32)
    cos2 = np.ascontiguousarray(np.concatenate([cosT, cosT], axis=0))
    sinS = np.ascontiguousarray(np.concatenate([-sinT, sinT], axis=0))

    p = np.arange(128)[:, None]
    tp = np.arange(TT)[None, :]
    # mask2[p, diag, hp, t] = p <= t - 128*diag, duplicated over hp
    mask2 = np.stack(
        [
            np.stack([(p <= tp - 128 * j).astype(np.float32)] * 2, axis=1)
            for j in range(4)
        ],
        axis=1,
    )  # [128, 4, 2, TT]
    mask2 = np.ascontiguousarray(mask2).astype(bf16)
    ones = np.ones((128, 128), dtype=bf16)
    ident = np.eye(128, dtype=np.float32).astype(bf16)

    Wq = np.asarray(Wq, dtype=np.float32)
    Wk = np.asarray(Wk, dtype=np.float32)
    Wv = np.asarray(Wv, dtype=np.float32)
    Wo = np.asarray(Wo, dtype=np.float32)

    in_maps = []
    for c in range(N_CORES):
        qs = slice(HQ * D * c, HQ * D * (c + 1))
        ks = slice(D * c, D * (c + 1))
        in_maps.append(
            {
                "xT": xT,
                "wqkvT": np.ascontiguousarray(
                    np.concatenate(
                        [Wq[qs, :].T, Wk[ks, :].T, Wv[ks, :].T], axis=1
                    )
                ).astype(bf16),
                "woT": np.ascontiguousarray(Wo[qs, :].T).astype(bf16),
                "cos2": cos2.astype(bf16),
                "sinS": sinS.astype(bf16),
                "mask2": mask2,
                "ones_i": ones,
                "ident_i": ident,
            }
        )
    return in_maps


def get_program():
    global _cached
    if _cached is None:
        _cached = _build()
    return _cached


def kernel(hidden_states, Wq, Wk, Wv, Wo):
    nc = get_program()
    in_maps = _host_inputs(hidden_states, Wq, Wk, Wv, Wo)
    res = run_bass_kernel_spmd(nc, in_maps, list(range(N_CORES)))
    outT = np.concatenate([res.results[c]["out"] for c in range(N_CORES)], axis=0)
    return np.ascontiguousarray(outT.T).reshape(1, T, HID).astype(np.float32)


# revision 38
# speedup vs baseline: 1.0679x; 1.0009x over previous
"""Trainium2 Bass kernel for GQA attention layer (B=1, T=2048, HID=4096,
32 q-heads / 8 kv-heads, head_dim 128, RoPE, causal) sharded over 8 cores.

Sharding: tensor-parallel over heads. Core c owns q-heads 4c..4c+3 and
kv-head c. Per-head attention outputs are AllGathered eagerly (16 chunks);
each core then computes a 512-row slice of the output projection over the
full 4096 hd dims, so no AllReduce is needed. Host assembles the slices.

Structure (vs previous version):
- projection runs in three passes (q, k, v) so the RoPE DVE chain hides
  under the v pass instead of stalling the first score matmuls, and the
  k accumulator's PSUM WAR wait resolves under the q pass
- attention processes q-heads in pairs: one fused exp per k-block spanning
  both heads' score banks ([128,2,512] PSUM tiles), softmax denominators
  via two col-tiled (M=32) matmuls that co-run in distinct PE col groups
- AllGather is per-head and eager; outproj phases are slotted between
  proj/attn phases to cover rope latency (outproj3 is head-major so it
  consumes gather chunks in arrival order)
"""

import numpy as np

import concourse.bacc as bacc
import concourse.mybir as mybir
import concourse.tile as tile
from concourse.bass_utils import run_bass_kernel_spmd
from concourse.tile import add_dep_helper

T = 2048
HID = 4096
D = 128
N_HEADS = 32
N_KV = 8
HQ = N_HEADS // N_KV  # q heads per core (=4)
TT = 512  # t tile
NTT = T // TT  # 4
NH = HID // 128  # 32 h-tiles
SCALE = 1.0 / np.sqrt(np.float32(D))
ROPE_BASE = 10000.0
N_CORES = 8

_F32 = mybir.dt.float32
_DT = mybir.dt.bfloat16

_cached = None


def _build():
    nc = bacc.Bacc("TRN2", target_bir_lowering=False, debug=False, num_devices=N_CORES)

    xT = nc.dram_tensor("xT", [HID, T], _DT, kind="ExternalInput").ap()
    wqkvT = nc.dram_tensor(
        "wqkvT", [HID, (HQ + 2) * D], _DT, kind="ExternalInput"
    ).ap()
    woT = nc.dram_tensor("woT", [HID, HQ * D], _DT, kind="ExternalInput").ap()
    cos2 = nc.dram_tensor("cos2", [128, T], _DT, kind="ExternalInput").ap()
    sinS = nc.dram_tensor("sinS", [128, T], _DT, kind="ExternalInput").ap()
    mask2 = nc.dram_tensor("mask2", [128, 4, 2, TT], _DT, kind="ExternalInput").ap()
    ones_i = nc.dram_tensor("ones_i", [128, 128], _DT, kind="ExternalInput").ap()
    ident_i = nc.dram_tensor("ident_i", [128, 128], _DT, kind="ExternalInput").ap()
    out = nc.dram_tensor("out", [HQ * D, T], _F32, kind="ExternalOutput").ap()

    Exp = mybir.ActivationFunctionType.Exp

    with tile.TileContext(nc) as tc:
        with (
            tc.tile_pool(name="const", bufs=1) as const,
            tc.tile_pool(name="big", bufs=1) as big,
            tc.tile_pool(name="sb", bufs=1) as sb,
            tc.tile_pool(name="ps", bufs=1, space="PSUM") as ps,
            tc.tile_pool(name="dram", bufs=1, space="DRAM") as dram,
        ):
            # ---- constants / persistent weights in SBUF ----
            cos_sb = const.tile([128, T], _DT, name="cos_sb")
            sin_sb = const.tile([128, T], _DT, name="sin_sb")
            # mask_sb[p, diag, hp, t_local]
            mask_sb = const.tile([128, 4, 2, TT], _DT, name="mask_sb")
            ones_sb = const.tile([128, 128], _DT, name="ones_sb")
            ident_sb = const.tile([128, 128], _DT, name="ident_sb")
            wqkv_t = [
                const.tile([128, (HQ + 2) * D], _DT, name=f"wqkv_t{j}")
                for j in range(NH)
            ]
            wo_sb = const.tile([128, NH * HQ * D], _DT, name="wo_sb")

            def emit_consts():
                # emitted after proj(0)'s weight/x DMA stream so the first
                # matmuls aren't queued behind 2MB of constants; all of these
                # are ungated so they drain without blocking their queues
                nc.scalar.dma_start(out=ident_sb[:], in_=ident_i[:])
                nc.scalar.dma_start(out=cos_sb[:], in_=cos2[:])
                nc.gpsimd.dma_start(out=sin_sb[:], in_=sinS[:])
                nc.gpsimd.dma_start(out=ones_sb[:], in_=ones_i[:])
                nc.gpsimd.dma_start(out=mask_sb[:], in_=mask2[:])

            qrot = [big.tile([128, T], _DT, name=f"qrot{h}") for h in range(HQ)]
            krot = big.tile([128, T], _DT, name="krot")
            v_sb = big.tile([128, T], _DT, name="v_sb")  # V[s,d]: block k at cols 128k

            attn_local = [
                dram.tile([HQ * D, TT], _DT, name=f"attn_local{i}") for i in range(NTT)
            ]
            # per-tile gathered activations: [core*512 rows, TT]
            attn_full = [
                dram.tile(
                    [N_CORES * HQ * D, TT],
                    _DT,
                    addr_space="Shared",
                    name=f"attn_full{i}",
                )
                for i in range(NTT)
            ]
            # tile 3 is gathered in two half-chunks so outproj(3) can start
            # on heads 0-1 while heads 2-3 are still in flight
            attn_half = [
                dram.tile(
                    [N_CORES * 2 * D, TT], _DT, addr_space="Shared", name=f"attn_h{i}"
                )
                for i in range(2)
            ]

            # PE warmup: dummy matmuls on (not yet loaded) const tiles fill
            # the DMA-bound first ~10us and flip the HAM clock gate to 8/8
            # before the real matmuls arrive. Results are discarded.
            warm = ps.tile([128, 2, TT], _F32, tag="sc", bufs=2, name="warm")
            for _ in range(48):
                nc.tensor.matmul(
                    warm[:, 0, :],
                    cos_sb[:, 0:128],
                    cos_sb[:, 512:1024],
                    start=True,
                    stop=True,
                )

            def proj(ti, post_dma=None):
                tsl = slice(TT * ti, TT * (ti + 1))
                q01 = ps.tile([128, 2, TT], _F32, tag="sc", bufs=2, name=f"q01_{ti}")
                q23 = ps.tile([128, 2, TT], _F32, tag="sc", bufs=2, name=f"q23_{ti}")
                k_ps = ps.tile([128, TT], _F32, tag="at0", name=f"k_ps{ti}")
                xts = []
                # ---- q pass (4 MMs per h-tile) ----
                for hi in range(NH):
                    hsl = slice(128 * hi, 128 * (hi + 1))
                    # weight preloads are ungated: they drain early without
                    # blocking the scalar queue ahead of the attention exps
                    weng = nc.gpsimd if hi % 2 == 0 else nc.scalar
                    if ti == 0:
                        weng.dma_start(out=wqkv_t[hi][:], in_=wqkvT[hsl, :])
                    elif ti == 1:
                        weng.dma_start(
                            out=wo_sb[:, 512 * hi : 512 * (hi + 1)], in_=woT[hsl, :]
                        )
                    xt = sb.tile([128, TT], _DT, tag="x", bufs=32)
                    # x tiles are slot-WAR gated: keep them all on sync so a
                    # waiting head-of-queue DMA never starves the exps
                    nc.sync.dma_start(out=xt[:], in_=xT[hsl, tsl])
                    xts.append(xt)
                    st, sp = hi == 0, hi == NH - 1
                    for h in range(HQ):
                        nc.tensor.matmul(
                            (q01 if h < 2 else q23)[:, h % 2, :],
                            wqkv_t[hi][:, 128 * h : 128 * (h + 1)],
                            xt[:],
                            start=st,
                            stop=sp,
                        )
                if post_dma is not None:
                    post_dma()
                # ---- k pass ----
                for hi in range(NH):
                    nc.tensor.matmul(
                        k_ps[:],
                        wqkv_t[hi][:, HQ * D : (HQ + 1) * D],
                        xts[hi][:],
                        start=hi == 0,
                        stop=hi == NH - 1,
                    )

                # ---- RoPE (DVE + swap DMAs), overlapped with the next
                # outproj phase ----
                order = [HQ, 0, 1, 2, 3]  # k first: scores h0 need krot+qrot0
                stage = {}

                def src_of(h):
                    if h == HQ:
                        return k_ps[:]
                    return (q01 if h < 2 else q23)[:, h % 2, :]

                def rope_front(h):
                    qf = sb.tile([128, TT], _DT, tag="qf", bufs=5)
                    nc.vector.tensor_copy(qf[:], src_of(h))
                    qs = sb.tile([128, TT], _DT, tag="qs", bufs=5)
                    # gpsimd queue carries no bulk traffic -> low latency
                    nc.gpsimd.dma_start(out=qs[0:64, :], in_=qf[64:128, :])
                    nc.gpsimd.dma_start(out=qs[64:128, :], in_=qf[0:64, :])
                    t1 = sb.tile([128, TT], _DT, tag="t1", bufs=5)
                    nc.vector.tensor_mul(t1[:], qf[:], cos_sb[:, tsl])
                    stage[h] = (qs, t1)

                def rope_back(h):
                    qs, t1 = stage.pop(h)
                    t2 = sb.tile([128, TT], _DT, tag="t2", bufs=2)
                    nc.vector.tensor_mul(t2[:], qs[:], sin_sb[:, tsl])
                    dst = qrot[h][:, tsl] if h < HQ else krot[:, tsl]
                    nc.vector.tensor_add(dst, t1[:], t2[:])

                # all fronts first: batches the casts + swap DMAs so the
                # engine->DMA->engine latency is paid once, not per head
                for h in order:
                    rope_front(h)
                for h in order:
                    rope_back(h)

                # ---- v pass + transpose to [s,d] blocks (emitted after the
                # rope so the rope's DVE ops win scheduler priority) ----
                vT_ps = ps.tile([128, TT], _F32, tag="den", name=f"vT_ps{ti}")
                for hi in range(NH):
                    nc.tensor.matmul(
                        vT_ps[:],
                        wqkv_t[hi][:, (HQ + 1) * D : (HQ + 2) * D],
                        xts[hi][:],
                        start=hi == 0,
                        stop=hi == NH - 1,
                    )
                vT_sb = sb.tile([128, TT], _DT, tag="vTs", bufs=1)
                nc.vector.tensor_copy(vT_sb[:], vT_ps[:])
                for j in range(TT // 128):
                    vtr = ps.tile([128, 128], _DT, tag="tr", bufs=1)
                    nc.tensor.transpose(
                        vtr[:], vT_sb[:, 128 * j : 128 * (j + 1)], ident_sb[:]
                    )
                    k = (TT // 128) * ti + j
                    nc.vector.tensor_copy(v_sb[:, 128 * k : 128 * (k + 1)], vtr[:])

            def attn(ti):
                nblk = (TT // 128) * (ti + 1)
                last_exp = [None]
                for pair in range(2):
                    heads = (2 * pair, 2 * pair + 1)
                    at = [
                        ps.tile(
                            [128, TT], _F32, tag=f"at{hp}", name=f"at{ti}_{pair}{hp}"
                        )
                        for hp in range(2)
                    ]
                    den = ps.tile([128, TT], _F32, tag="den", name=f"den{ti}_{pair}")
                    def lo_of(k):
                        diag = k - (TT // 128) * ti
                        return 128 * diag if diag > 0 else 0

                    def emit_sc(k):
                        # scores for both heads + fused exp (+ causal mask)
                        diag = k - (TT // 128) * ti
                        lo = lo_of(k)
                        qsl = slice(TT * ti + lo, TT * (ti + 1))
                        ksl = slice(128 * k, 128 * (k + 1))
                        sc2 = ps.tile(
                            [128, 2, TT], _F32, tag="sc", bufs=2, name=f"sc{ti}_{k}"
                        )
                        for hp, h in enumerate(heads):
                            nc.tensor.matmul(
                                sc2[:, hp, lo:TT],
                                krot[:, ksl],
                                qrot[h][:, qsl],
                                start=True,
                                stop=True,
                            )
                        probs2 = sb.tile([128, 2, TT], _DT, tag="pr", bufs=5)
                        if diag >= 0:
                            ptmp = sb.tile([128, 2, TT], _DT, tag="pt", bufs=2)
                            ex = nc.scalar.activation(
                                ptmp[:, :, lo:TT], sc2[:, :, lo:TT], Exp, scale=SCALE
                            )
                            nc.vector.tensor_mul(
                                probs2[:, :, lo:TT],
                                ptmp[:, :, lo:TT],
                                mask_sb[:, diag, :, lo:TT],
                            )
                        else:
                            ex = nc.scalar.activation(
                                probs2[:, :, lo:TT], sc2[:, :, lo:TT], Exp, scale=SCALE
                            )
                        last_exp[0] = ex
                        return probs2

                    # 2-block software pipeline: scores+exp run ahead of pv/den
                    probs_t = {}
                    for k in range(min(2, nblk)):
                        probs_t[k] = emit_sc(k)
                    for k in range(nblk):
                        if k + 2 < nblk:
                            probs_t[k + 2] = emit_sc(k + 2)
                        probs2 = probs_t.pop(k)
                        lo = lo_of(k)
                        ksl = slice(128 * k, 128 * (k + 1))
                        st, sp = k == 0, k == nblk - 1
                        for hp, h in enumerate(heads):
                            nc.tensor.matmul(
                                den[32 * hp : 32 * (hp + 1), lo:TT],
                                ones_sb[:, 0:32],
                                probs2[:, hp, lo:TT],
                                start=st,
                                stop=sp,
                                tile_position=(0, 32 * hp),
                                # interp's zero-region group check rejects two
                                # col-tiled groups in one bank; HW-verified OK
                                skip_group_check=True,
                            )
                        for hp, h in enumerate(heads):
                            nc.tensor.matmul(
                                at[hp][:, lo:TT],
                                v_sb[:, ksl],
                                probs2[:, hp, lo:TT],
                                start=st,
                                stop=sp,
                            )
                    # normalize + store + eager per-head AllGather
                    # full-partition recip of the den bank (rows outside the
                    # pair's 32-row slices are garbage and never read)
                    r1 = sb.tile([128, TT], _F32, tag="r1", bufs=1)
                    nc.vector.reciprocal_approx_fast(r1[:, :], den[:, :])
                    for hp, h in enumerate(heads):
                        psl = slice(32 * hp, 32 * hp + 1)
                        # partition_broadcast only works from partition 0:
                        # hp=0 sits there already; hp=1 hops down via DMA
                        if hp > 0:
                            r0 = sb.tile([128, TT], _F32, tag="r0", bufs=2)
                            nc.gpsimd.dma_start(out=r0[0:1, :], in_=r1[psl, :])
                            bsrc = r0[0:1, :]
                        else:
                            bsrc = r1[0:1, :]
                        rB = sb.tile([128, TT], _F32, tag="rB", bufs=2)
                        nc.gpsimd.partition_broadcast(rB[:], bsrc, channels=128)
                        anorm = sb.tile([128, TT], _DT, tag="an", bufs=1)
                        nc.vector.tensor_mul(anorm[:], at[hp][:], rB[:])
                        nc.gpsimd.dma_start(
                            out=attn_local[ti][128 * h : 128 * (h + 1), :],
                            in_=anorm[:],
                        )
                    if ti == NTT - 1:
                        # last tile: gather each pair-half immediately
                        nc.gpsimd.collective_compute(
                            "AllGather",
                            mybir.AluOpType.bypass,
                            replica_groups=[list(range(N_CORES))],
                            ins=[attn_local[ti][256 * pair : 256 * (pair + 1), :]],
                            outs=[attn_half[pair].opt()],
                        )
                if ti < NTT - 1:
                    nc.gpsimd.collective_compute(
                        "AllGather",
                        mybir.AluOpType.bypass,
                        replica_groups=[list(range(N_CORES))],
                        ins=[attn_local[ti].opt()],
                        outs=[attn_full[ti].opt()],
                    )
                return last_exp[0]

            def outproj(ti, h_major=False, delay_after=None):
                if h_major:
                    order = [4 * r + h for h in range(HQ) for r in range(N_CORES)]
                else:
                    order = list(range(NH))
                o01 = ps.tile([128, 2, TT], _F32, tag="sc", bufs=2, name=f"o01_{ti}")
                o23 = ps.tile([128, 2, TT], _F32, tag="sc", bufs=2, name=f"o23_{ti}")
                for idx, j in enumerate(order):
                    r, h = j // HQ, j % HQ
                    ag = sb.tile([128, TT], _DT, tag="ag", bufs=8)
                    if h_major:
                        row = 256 * r + 128 * (h % 2)
                        src = attn_half[h // 2][row : row + 128, :]
                    else:
                        src = attn_full[ti][128 * j : 128 * (j + 1), :]
                    # split the gathered-activation stream over two queues
                    eng = nc.sync if idx % 2 == 0 else nc.scalar
                    dma = eng.dma_start(out=ag[:], in_=src)
                    if idx < 2 and delay_after is not None:
                        # don't let the scheduler hoist the collective-gated
                        # reads ahead of earlier phases' queue traffic
                        add_dep_helper(
                            dma.ins, delay_after.ins, sync=True, reason="ag-delay"
                        )
                    st, sp = idx == 0, idx == NH - 1
                    for o in range(4):
                        nc.tensor.matmul(
                            (o01 if o < 2 else o23)[:, o % 2, :],
                            wo_sb[:, 512 * j + 128 * o : 512 * j + 128 * (o + 1)],
                            ag[:],
                            start=st,
                            stop=sp,
                        )
                for o in range(4):
                    oc = sb.tile([128, TT], _F32, tag="oc", bufs=2)
                    nc.vector.tensor_copy(oc[:], (o01 if o < 2 else o23)[:, o % 2, :])
                    nc.gpsimd.dma_start(
                        out=out[128 * o : 128 * (o + 1), TT * ti : TT * (ti + 1)],
                        in_=oc[:],
                    )

            proj(0, emit_consts)
            attn(0)
            proj(1)
            e1 = attn(1)
            proj(2)
            outproj(0, delay_after=e1)
            e2 = attn(2)
            proj(3)
            outproj(1, delay_after=e2)
            attn(3)
            outproj(2)
            outproj(3, h_major=True)

    nc.compile()
    return nc


def _host_inputs(hidden_states, Wq, Wk, Wv, Wo):
    import ml_dtypes

    bf16 = ml_dtypes.bfloat16
    x = np.asarray(hidden_states, dtype=np.float32).reshape(T, HID)
    xT = np.ascontiguousarray(x.T).astype(bf16)

    pos = np.arange(T, dtype=np.float32)
    inv_freq = ROPE_BASE ** (-np.arange(0, D, 2, dtype=np.float32) / D)  # [64]
    ang = pos[:, None] * inv_freq[None, :]  # [T, 64]
    cosT = np.cos(ang).T.astype(np.float32)  # [64, T]
    sinT = np.sin(ang).T.astype(np.float32)
    cos2 = np.ascontiguousarray(np.concatenate([cosT, cosT], axis=0))
    sinS = np.ascontiguousarray(np.concatenate([-sinT, sinT], axis=0))

    p = np.arange(128)[:, None]
    tp = np.arange(TT)[None, :]
    # mask2[p, diag, hp, t] = p <= t - 128*diag, duplicated over hp
    mask2 = np.stack(
        [
            np.stack([(p <= tp - 128 * j).astype(np.float32)] * 2, axis=1)
            for j in range(4)
        ],
        axis=1,
    )  # [128, 4, 2, TT]
    mask2 = np.ascontiguousarray(mask2).astype(bf16)
    ones = np.ones((128, 128), dtype=bf16)
    ident = np.eye(128, dtype=np.float32).astype(bf16)

    Wq = np.asarray(Wq, dtype=np.float32)
    Wk = np.asarray(Wk, dtype=np.float32)
    Wv = np.asarray(Wv, dtype=np.float32)
    Wo = np.asarray(Wo, dtype=np.float32)

    in_maps = []
    for c in range(N_CORES):
        qs = slice(HQ * D * c, HQ * D * (c + 1))
        ks = slice(D * c, D * (c + 1))
        in_maps.append(
            {
                "xT": xT,
                "wqkvT": np.ascontiguousarray(
                    np.concatenate(
                        [Wq[qs, :].T, Wk[ks, :].T, Wv[ks, :].T], axis=1
                    )
                ).astype(bf16),
                "woT": np.ascontiguousarray(Wo[qs, :].T).astype(bf16),
                "cos2": cos2.astype(bf16),
                "sinS": sinS.astype(bf16),
                "mask2": mask2,
                "ones_i": ones,
                "ident_i": ident,
            }
        )
    return in_maps


def get_program():
    global _cached
    if _cached is None:
        _cached = _build()
    return _cached


def kernel(hidden_states, Wq, Wk, Wv, Wo):
    nc = get_program()
    in_maps = _host_inputs(hidden_states, Wq, Wk, Wv, Wo)
    res = run_bass_kernel_spmd(nc, in_maps, list(range(N_CORES)))
    outT = np.concatenate([res.results[c]["out"] for c in range(N_CORES)], axis=0)
    return np.ascontiguousarray(outT.T).reshape(1, T, HID).astype(np.float32)
